# revision 4
# baseline (speedup 1.0000x reference)
"""Criss-cross self-attention on 8 Trainium2 NeuronCores.

kernel(**inputs) takes the FULL inputs (x, Wq, Wk, Wv, Wo) and returns the
full output of the reference nn.Module.

Sharding: core = b * 2 + g, where b in [0,4) is the batch index and g in
[0,2) selects a group of 4 heads (tensor-parallel over heads).  Each core:
  stage A: projects q/k/v for its 256 channels   (qkv = waT.T @ x)
  stage B: vertical + horizontal axial attention for its 4 heads
  stage C: partial output projection  y_part = Wo[:, local cols] @ out_local
Host sums the two partial y's per batch element.
"""

import sys

sys.path.insert(0, "/opt/trn_rl_repo")

import numpy as np
import concourse.bass as bass
import concourse.mybir as mybir
from concourse import tile
from concourse.bass_utils import run_bass_kernel_spmd
from concourse.vector_clock import ScopedClock, VectorClock

FP32 = mybir.dt.float32
BF16 = mybir.dt.bfloat16
AF = mybir.ActivationFunctionType

C = 512
H = 128
W = 128
B = 4
S = H * W  # 16384
N_CORES = 8


MAX_WAITS = 1  # this walrus build rejects multiple sem waits on an instruction


class PatchedTileContext(tile.TileContext):
    """Work around 'Too many sync wait commands' in this walrus build:
    - the tile tail-drain gets one NOP per outstanding proc
    - any instruction with more than MAX_WAITS sem waits gets the excess
      moved onto same-engine NOPs inserted immediately before it (engine
      queues are FIFO, so blocking the queue on the NOP is equivalent)."""

    _nop_seq = 0

    def _split_excess_waits(self, ordered):
        for bb_name, insts in ordered.items():
            out = []
            for inst in insts:
                si = inst.sync_info
                waits = list(si.on_wait) if si is not None and si.on_wait else []
                if len(waits) > MAX_WAITS and inst.engine is not None:
                    keep = waits[:MAX_WAITS]
                    rest = waits[MAX_WAITS:]
                    while rest:
                        chunk, rest = rest[:MAX_WAITS], rest[MAX_WAITS:]
                        PatchedTileContext._nop_seq += 1
                        nop = mybir.InstNoOp(
                            name=f"I-waitsplit-{PatchedTileContext._nop_seq}",
                            ins=[],
                            outs=[],
                        )
                        nop.engine = inst.engine
                        nop.bass_nofuse = True
                        nop.sync_info = mybir.SyncInfo(on_wait=chunk, on_update=[])
                        out.append(nop)
                    inst.sync_info = mybir.SyncInfo(
                        on_wait=keep,
                        on_update=list(si.on_update) if si.on_update else [],
                    )
                out.append(inst)
            ordered[bb_name] = out
        return ordered

    def _lower_ordered_insts(self, ordered):
        super()._lower_ordered_insts(self._split_excess_waits(ordered))

    def _drain_and_barrier(self, tick_clock, wait_clock):
        nc = self.nc
        gc = tick_clock.global_clock
        n = len(gc)
        for proc in range(n):
            t = gc[proc]
            if t > 0:
                nop_inst = nc.sync.nop(nofuse=True)
                vc = VectorClock([t if i == proc else 0 for i in range(n)])
                wait_clock.add_sem_waits(nop_inst.ins, ScopedClock({None: vc}))
        nc.sync.drain()
        nc.all_engine_barrier()
        popped = nc._tile_sem_poison_stack.pop()
        assert popped is self._sem_poison
        nc.clear_and_free_semaphores(list(self.sems.allocated().values()))
        nc.all_engine_barrier()


def build_nc():
    nc = bass.Bass()
    x = nc.declare_dram_parameter("x", [C, S], BF16, isOutput=False)
    waT = nc.declare_dram_parameter("waT", [C, 768], BF16, isOutput=False)
    woT = nc.declare_dram_parameter("woT", [C, C], BF16, isOutput=False)
    identity = nc.declare_dram_parameter("ident", [128, 128], BF16, isOutput=False)
    y = nc.declare_dram_parameter("y", [C, S], FP32, isOutput=True)

    qkv = nc.dram_tensor("qkv", [768, S], BF16)
    outb = nc.dram_tensor("outb", [C, S], BF16)

    NB = 2048  # free-dim block for the dense matmul stages

    with PatchedTileContext(nc) as tc:
        with tc.tile_pool(name="const", bufs=1) as constp:
            ident = constp.tile([128, 128], BF16)
            nc.sync.dma_start(out=ident[:], in_=identity[:])

            # ---------------- stage A: qkv = waT.T @ x ----------------
            with (
                tc.tile_pool(name="wa", bufs=1) as wap,
                tc.tile_pool(name="xblk", bufs=2) as xp,
                tc.tile_pool(name="apsum", bufs=4, space="PSUM") as app,
                tc.tile_pool(name="aout", bufs=3) as aop,
            ):
                wa_t = []
                for k4 in range(4):
                    t = wap.tile([128, 768], BF16, tag=f"wa{k4}")
                    nc.sync.dma_start(out=t[:], in_=waT[k4 * 128 : (k4 + 1) * 128, :])
                    wa_t.append(t)
                for nb in range(S // NB):
                    xt = []
                    for k4 in range(4):
                        t = xp.tile([128, NB], BF16, tag=f"x{k4}")
                        nc.sync.dma_start(
                            out=t[:],
                            in_=x[k4 * 128 : (k4 + 1) * 128, nb * NB : (nb + 1) * NB],
                        )
                        xt.append(t)
                    for m in range(6):
                        ot = aop.tile([128, NB], BF16, tag="ao")
                        for n2 in range(NB // 512):
                            ps = app.tile([128, 512], FP32, tag="aps")
                            for k4 in range(4):
                                nc.tensor.matmul(
                                    ps[:],
                                    lhsT=wa_t[k4][:, m * 128 : (m + 1) * 128],
                                    rhs=xt[k4][:, n2 * 512 : (n2 + 1) * 512],
                                    start=(k4 == 0),
                                    stop=(k4 == 3),
                                )
                            # alternate evict engine per output tile so the
                            # trailing DMA waits on a single engine sem
                            if (nb * 6 + m) % 2 == 0:
                                nc.scalar.copy(
                                    out=ot[:, n2 * 512 : (n2 + 1) * 512], in_=ps[:]
                                )
                            else:
                                nc.vector.tensor_copy(
                                    ot[:, n2 * 512 : (n2 + 1) * 512], ps[:]
                                )
                        nc.sync.dma_start(
                            out=qkv[m * 128 : (m + 1) * 128, nb * NB : (nb + 1) * NB],
                            in_=ot[:],
                        )

            # ---------------- stage B: axial attention ----------------
            for p in range(2):
                with (
                    tc.tile_pool(name=f"qkv{p}", bufs=1) as qp,
                    tc.tile_pool(name=f"outsb{p}", bufs=2) as outp,
                    tc.tile_pool(name=f"bps{p}", bufs=2, space="PSUM") as bpp,
                    tc.tile_pool(name=f"bsb{p}", bufs=3) as bsp,
                ):
                    q_sb = qp.tile([128, S], BF16, tag="q")
                    k_sb = qp.tile([128, S], BF16, tag="k")
                    v_sb = qp.tile([128, S], BF16, tag="v")
                    nc.sync.dma_start(out=q_sb[:], in_=qkv[p * 128 : (p + 1) * 128, :])
                    nc.sync.dma_start(
                        out=k_sb[:], in_=qkv[256 + p * 128 : 256 + (p + 1) * 128, :]
                    )
                    nc.sync.dma_start(
                        out=v_sb[:], in_=qkv[512 + p * 128 : 512 + (p + 1) * 128, :]
                    )
                    for d in range(2):  # 0 = vertical (fixed w), 1 = horizontal
                        out_sb = outp.tile([128, S], BF16, tag="out")

                        def sl(t, lo, hi, s):
                            if d == 0:
                                return t[lo:hi, s::W]
                            return t[lo:hi, s * W : (s + 1) * W]

                        for s in range(128):
                            tp_ps = bpp.tile([128, 128], BF16, tag="tp")
                            nc.tensor.transpose(tp_ps[:], sl(v_sb, 0, 128, s), ident[:])
                            vt = bsp.tile([128, 128], BF16, tag="vt")
                            nc.vector.tensor_copy(vt[:], tp_ps[:])
                            for head in range(2):
                                cb = head * 64
                                sc_ps = bpp.tile([128, 128], FP32, tag="sc")
                                nc.tensor.matmul(
                                    sc_ps[:],
                                    lhsT=sl(q_sb, cb, cb + 64, s),
                                    rhs=sl(k_sb, cb, cb + 64, s),
                                    start=True,
                                    stop=True,
                                )
                                e_sb = bsp.tile([128, 128], BF16, tag="e")
                                den = bsp.tile([128, 1], FP32, tag="den")
                                nc.scalar.activation(
                                    e_sb[:], sc_ps[:], AF.Exp, accum_out=den[:]
                                )
                                rec = bsp.tile([128, 1], FP32, tag="rec")
                                nc.vector.reciprocal(rec[:], den[:])
                                en = bsp.tile([128, 128], BF16, tag="en")
                                nc.gpsimd.tensor_scalar_mul(en[:], e_sb[:], rec[:])
                                et_ps = bpp.tile([128, 128], BF16, tag="et")
                                nc.tensor.transpose(et_ps[:], en[:], ident[:])
                                et = bsp.tile([128, 128], BF16, tag="ets")
                                nc.scalar.copy(out=et[:], in_=et_ps[:])
                                av_ps = bpp.tile([64, 128], FP32, tag="av")
                                nc.tensor.matmul(
                                    av_ps[:],
                                    lhsT=vt[:, cb : cb + 64],
                                    rhs=et[:],
                                    start=True,
                                    stop=True,
                                )
                                nc.vector.tensor_copy(
                                    sl(out_sb, cb, cb + 64, s), av_ps[:]
                                )
                        nc.sync.dma_start(
                            out=outb[d * 256 + p * 128 : d * 256 + (p + 1) * 128, :],
                            in_=out_sb[:],
                        )

            # ---------------- stage C: y = woT.T @ outb ----------------
            with (
                tc.tile_pool(name="wo", bufs=1) as wop,
                tc.tile_pool(name="oblk", bufs=2) as obp,
                tc.tile_pool(name="cpsum", bufs=4, space="PSUM") as cpp,
                tc.tile_pool(name="yout", bufs=3) as yp,
            ):
                wo_t = []
                for k4 in range(4):
                    t = wop.tile([128, 512], BF16, tag=f"wo{k4}")
                    nc.sync.dma_start(out=t[:], in_=woT[k4 * 128 : (k4 + 1) * 128, :])
                    wo_t.append(t)
                for nb in range(S // NB):
                    ot = []
                    for k4 in range(4):
                        t = obp.tile([128, NB], BF16, tag=f"ob{k4}")
                        nc.sync.dma_start(
                            out=t[:],
                            in_=outb[
                                k4 * 128 : (k4 + 1) * 128, nb * NB : (nb + 1) * NB
                            ],
                        )
                        ot.append(t)
                    for m in range(4):
                        yt = yp.tile([128, NB], FP32, tag="yt")
                        for n2 in range(NB // 512):
                            ps = cpp.tile([128, 512], FP32, tag="cps")
                            for k4 in range(4):
                                nc.tensor.matmul(
                                    ps[:],
                                    lhsT=wo_t[k4][:, m * 128 : (m + 1) * 128],
                                    rhs=ot[k4][:, n2 * 512 : (n2 + 1) * 512],
                                    start=(k4 == 0),
                                    stop=(k4 == 3),
                                )
                            if (nb * 4 + m) % 2 == 0:
                                nc.scalar.copy(
                                    out=yt[:, n2 * 512 : (n2 + 1) * 512], in_=ps[:]
                                )
                            else:
                                nc.vector.tensor_copy(
                                    yt[:, n2 * 512 : (n2 + 1) * 512], ps[:]
                                )
                        nc.sync.dma_start(
                            out=y[m * 128 : (m + 1) * 128, nb * NB : (nb + 1) * NB],
                            in_=yt[:],
                        )
    return nc


def make_in_maps(x, Wq, Wk, Wv, Wo):
    """Build the 8 per-core input maps from the full-problem inputs."""
    import ml_dtypes

    x = np.asarray(x, dtype=np.float32).reshape(B, C, S)
    Wq = np.asarray(Wq, np.float32)
    Wk = np.asarray(Wk, np.float32)
    Wv = np.asarray(Wv, np.float32)
    Wo = np.asarray(Wo, np.float32)
    ident = np.eye(128, dtype=np.float32)

    def bf16(a):
        return np.ascontiguousarray(a).astype(ml_dtypes.bfloat16)

    in_maps = []
    for core in range(N_CORES):
        b, g = divmod(core, 2)
        lo, hi = g * 256, (g + 1) * 256
        wa = np.concatenate([Wq[lo:hi], Wk[lo:hi], Wv[lo:hi]], axis=0).T.copy()
        wo_loc = np.concatenate(
            [Wo[:, lo:hi].T, Wo[:, C + lo : C + hi].T], axis=0
        ).copy()
        in_maps.append(
            {
                "x": bf16(x[b]),
                "waT": bf16(wa),
                "woT": bf16(wo_loc),
                "ident": bf16(ident),
            }
        )
    return in_maps


def combine_results(results):
    """results: list of 8 dicts with 'y' (512, S) fp32 -> full (4,512,128,128)."""
    y = np.empty((B, C, H, W), np.float32)
    for b in range(B):
        y[b] = (
            results[2 * b]["y"].astype(np.float32)
            + results[2 * b + 1]["y"].astype(np.float32)
        ).reshape(C, H, W)
    return y


_NC_CACHE = None


def get_nc():
    global _NC_CACHE
    if _NC_CACHE is None:
        _NC_CACHE = build_nc()
    return _NC_CACHE


def kernel(x, Wq, Wk, Wv, Wo):
    nc = get_nc()
    in_maps = make_in_maps(x, Wq, Wk, Wv, Wo)
    res = run_bass_kernel_spmd(nc, in_maps, list(range(N_CORES)), trace=False)
    return combine_results(res.results)


# revision 6
# speedup vs baseline: 37.1302x; 37.1302x over previous
"""Criss-cross self-attention on 8 Trainium2 NeuronCores.

kernel(**inputs) takes the FULL inputs (x, Wq, Wk, Wv, Wo) and returns the
full output of the reference nn.Module.

Sharding: core = b * 2 + g, where b in [0,4) is the batch index and g in
[0,2) selects a group of 4 heads (tensor-parallel over heads).  Each core:
  stage A: projects q/k/v for its 256 channels   (qkv = waT.T @ x)
  stage B: vertical + horizontal axial attention for its 4 heads
  stage C: partial output projection  y_part = Wo[:, local cols] @ out_local
Host sums the two partial y's per batch element.
"""

import sys

sys.path.insert(0, "/opt/trn_rl_repo")

import numpy as np
import concourse.bass as bass
import concourse.mybir as mybir
from concourse import tile
from concourse.bass_utils import run_bass_kernel_spmd
from concourse.vector_clock import ScopedClock, VectorClock

FP32 = mybir.dt.float32
BF16 = mybir.dt.bfloat16
FP16 = mybir.dt.float16
AF = mybir.ActivationFunctionType

C = 512
H = 128
W = 128
B = 4
S = H * W  # 16384
N_CORES = 8


MAX_WAITS = 1  # this walrus build rejects multiple sem waits on an instruction


class PatchedTileContext(tile.TileContext):
    """Work around 'Too many sync wait commands' in this walrus build:
    - the tile tail-drain gets one NOP per outstanding proc
    - any instruction with more than MAX_WAITS sem waits gets the excess
      moved onto same-engine NOPs inserted immediately before it (engine
      queues are FIFO, so blocking the queue on the NOP is equivalent)."""

    _nop_seq = 0

    def _split_excess_waits(self, ordered):
        for bb_name, insts in ordered.items():
            out = []
            for inst in insts:
                si = inst.sync_info
                waits = list(si.on_wait) if si is not None and si.on_wait else []
                if len(waits) > MAX_WAITS and inst.engine is not None:
                    keep = waits[:MAX_WAITS]
                    rest = waits[MAX_WAITS:]
                    while rest:
                        chunk, rest = rest[:MAX_WAITS], rest[MAX_WAITS:]
                        PatchedTileContext._nop_seq += 1
                        nop = mybir.InstNoOp(
                            name=f"I-waitsplit-{PatchedTileContext._nop_seq}",
                            ins=[],
                            outs=[],
                        )
                        nop.engine = inst.engine
                        nop.bass_nofuse = True
                        nop.sync_info = mybir.SyncInfo(on_wait=chunk, on_update=[])
                        out.append(nop)
                    inst.sync_info = mybir.SyncInfo(
                        on_wait=keep,
                        on_update=list(si.on_update) if si.on_update else [],
                    )
                out.append(inst)
            ordered[bb_name] = out
        return ordered

    def _lower_ordered_insts(self, ordered):
        super()._lower_ordered_insts(self._split_excess_waits(ordered))

    def _drain_and_barrier(self, tick_clock, wait_clock):
        nc = self.nc
        gc = tick_clock.global_clock
        n = len(gc)
        for proc in range(n):
            t = gc[proc]
            if t > 0:
                nop_inst = nc.sync.nop(nofuse=True)
                vc = VectorClock([t if i == proc else 0 for i in range(n)])
                wait_clock.add_sem_waits(nop_inst.ins, ScopedClock({None: vc}))
        nc.sync.drain()
        nc.all_engine_barrier()
        popped = nc._tile_sem_poison_stack.pop()
        assert popped is self._sem_poison
        nc.clear_and_free_semaphores(list(self.sems.allocated().values()))
        nc.all_engine_barrier()


def build_nc(loop_iters=None):
    nc = bass.Bass()
    x = nc.declare_dram_parameter("x", [C, S], FP16, isOutput=False)
    waT = nc.declare_dram_parameter("waT", [C, 768], FP16, isOutput=False)
    woT = nc.declare_dram_parameter("woT", [C, C], FP16, isOutput=False)
    identity = nc.declare_dram_parameter("ident", [128, 128], FP16, isOutput=False)
    y = nc.declare_dram_parameter("y", [C, S], FP32, isOutput=True)

    qkv = nc.dram_tensor("qkv", [768, S], FP16)
    outb = nc.dram_tensor("outb", [C, S], FP16)

    NB = 2048  # free-dim block for the dense matmul stages

    from contextlib import nullcontext

    with PatchedTileContext(nc) as tc:
        loop_cm = tc.For_i(0, loop_iters, 1) if loop_iters else nullcontext()
        with loop_cm, tc.tile_pool(name="const", bufs=1) as constp:
            ident = constp.tile([128, 128], FP16)
            nc.sync.dma_start(out=ident[:], in_=identity[:])

            # ---------------- stage A: qkv = waT.T @ x ----------------
            with (
                tc.tile_pool(name="wa", bufs=1) as wap,
                tc.tile_pool(name="xblk", bufs=2) as xp,
                tc.tile_pool(name="apsum", bufs=4, space="PSUM") as app,
                tc.tile_pool(name="aout", bufs=3) as aop,
            ):
                wa_t = []
                for k4 in range(4):
                    t = wap.tile([128, 768], FP16, tag=f"wa{k4}")
                    nc.sync.dma_start(out=t[:], in_=waT[k4 * 128 : (k4 + 1) * 128, :])
                    wa_t.append(t)
                for nb in range(S // NB):
                    xt = []
                    for k4 in range(4):
                        t = xp.tile([128, NB], FP16, tag=f"x{k4}")
                        nc.sync.dma_start(
                            out=t[:],
                            in_=x[k4 * 128 : (k4 + 1) * 128, nb * NB : (nb + 1) * NB],
                        )
                        xt.append(t)
                    for m in range(6):
                        ot = aop.tile([128, NB], FP16, tag="ao")
                        for n2 in range(NB // 512):
                            ps = app.tile([128, 512], FP32, tag="aps")
                            for k4 in range(4):
                                nc.tensor.matmul(
                                    ps[:],
                                    lhsT=wa_t[k4][:, m * 128 : (m + 1) * 128],
                                    rhs=xt[k4][:, n2 * 512 : (n2 + 1) * 512],
                                    start=(k4 == 0),
                                    stop=(k4 == 3),
                                )
                            # alternate evict engine per output tile so the
                            # trailing DMA waits on a single engine sem
                            if (nb * 6 + m) % 2 == 0:
                                nc.scalar.copy(
                                    out=ot[:, n2 * 512 : (n2 + 1) * 512], in_=ps[:]
                                )
                            else:
                                nc.vector.tensor_copy(
                                    ot[:, n2 * 512 : (n2 + 1) * 512], ps[:]
                                )
                        nc.sync.dma_start(
                            out=qkv[m * 128 : (m + 1) * 128, nb * NB : (nb + 1) * NB],
                            in_=ot[:],
                        )

            # ---------------- stage B: axial attention ----------------
            for p in range(2):
                with (
                    tc.tile_pool(name=f"qkv{p}", bufs=1) as qp,
                    tc.tile_pool(name=f"outsb{p}", bufs=2) as outp,
                    tc.tile_pool(name=f"bps{p}", bufs=2, space="PSUM") as bpp,
                    tc.tile_pool(name=f"bsb{p}", bufs=3) as bsp,
                ):
                    q_sb = qp.tile([128, S], FP16, tag="q")
                    k_sb = qp.tile([128, S], FP16, tag="k")
                    v_sb = qp.tile([128, S], FP16, tag="v")
                    nc.sync.dma_start(out=q_sb[:], in_=qkv[p * 128 : (p + 1) * 128, :])
                    nc.sync.dma_start(
                        out=k_sb[:], in_=qkv[256 + p * 128 : 256 + (p + 1) * 128, :]
                    )
                    nc.sync.dma_start(
                        out=v_sb[:], in_=qkv[512 + p * 128 : 512 + (p + 1) * 128, :]
                    )
                    for d in range(2):  # 0 = vertical (fixed w), 1 = horizontal
                        out_sb = outp.tile([128, S], FP16, tag="out")

                        def sl(t, lo, hi, s):
                            if d == 0:
                                return t[lo:hi, s::W]
                            return t[lo:hi, s * W : (s + 1) * W]

                        for s in range(128):
                            tp_ps = bpp.tile([128, 128], FP16, tag="tp")
                            nc.tensor.transpose(tp_ps[:], sl(v_sb, 0, 128, s), ident[:])
                            vt = bsp.tile([128, 128], FP16, tag="vt")
                            nc.vector.tensor_copy(vt[:], tp_ps[:])
                            for head in range(2):
                                cb = head * 64
                                sc_ps = bpp.tile([128, 128], FP32, tag="sc")
                                nc.tensor.matmul(
                                    sc_ps[:],
                                    lhsT=sl(q_sb, cb, cb + 64, s),
                                    rhs=sl(k_sb, cb, cb + 64, s),
                                    start=True,
                                    stop=True,
                                )
                                e_sb = bsp.tile([128, 128], BF16, tag="e")
                                den = bsp.tile([128, 1], FP32, tag="den")
                                nc.scalar.activation(
                                    e_sb[:], sc_ps[:], AF.Exp, accum_out=den[:]
                                )
                                rec = bsp.tile([128, 1], FP32, tag="rec")
                                nc.vector.reciprocal(rec[:], den[:])
                                en = bsp.tile([128, 128], FP16, tag="en")
                                nc.gpsimd.tensor_scalar_mul(en[:], e_sb[:], rec[:])
                                et_ps = bpp.tile([128, 128], FP16, tag="et")
                                nc.tensor.transpose(et_ps[:], en[:], ident[:])
                                et = bsp.tile([128, 128], FP16, tag="ets")
                                nc.scalar.copy(out=et[:], in_=et_ps[:])
                                av_ps = bpp.tile([64, 128], FP32, tag="av")
                                nc.tensor.matmul(
                                    av_ps[:],
                                    lhsT=vt[:, cb : cb + 64],
                                    rhs=et[:],
                                    start=True,
                                    stop=True,
                                )
                                nc.vector.tensor_copy(
                                    sl(out_sb, cb, cb + 64, s), av_ps[:]
                                )
                        nc.sync.dma_start(
                            out=outb[d * 256 + p * 128 : d * 256 + (p + 1) * 128, :],
                            in_=out_sb[:],
                        )

            # ---------------- stage C: y = woT.T @ outb ----------------
            with (
                tc.tile_pool(name="wo", bufs=1) as wop,
                tc.tile_pool(name="oblk", bufs=2) as obp,
                tc.tile_pool(name="cpsum", bufs=4, space="PSUM") as cpp,
                tc.tile_pool(name="yout", bufs=3) as yp,
            ):
                wo_t = []
                for k4 in range(4):
                    t = wop.tile([128, 512], FP16, tag=f"wo{k4}")
                    nc.sync.dma_start(out=t[:], in_=woT[k4 * 128 : (k4 + 1) * 128, :])
                    wo_t.append(t)
                for nb in range(S // NB):
                    ot = []
                    for k4 in range(4):
                        t = obp.tile([128, NB], FP16, tag=f"ob{k4}")
                        nc.sync.dma_start(
                            out=t[:],
                            in_=outb[
                                k4 * 128 : (k4 + 1) * 128, nb * NB : (nb + 1) * NB
                            ],
                        )
                        ot.append(t)
                    for m in range(4):
                        yt = yp.tile([128, NB], FP32, tag="yt")
                        for n2 in range(NB // 512):
                            ps = cpp.tile([128, 512], FP32, tag="cps")
                            for k4 in range(4):
                                nc.tensor.matmul(
                                    ps[:],
                                    lhsT=wo_t[k4][:, m * 128 : (m + 1) * 128],
                                    rhs=ot[k4][:, n2 * 512 : (n2 + 1) * 512],
                                    start=(k4 == 0),
                                    stop=(k4 == 3),
                                )
                            if (nb * 4 + m) % 2 == 0:
                                nc.scalar.copy(
                                    out=yt[:, n2 * 512 : (n2 + 1) * 512], in_=ps[:]
                                )
                            else:
                                nc.vector.tensor_copy(
                                    yt[:, n2 * 512 : (n2 + 1) * 512], ps[:]
                                )
                        nc.sync.dma_start(
                            out=y[m * 128 : (m + 1) * 128, nb * NB : (nb + 1) * NB],
                            in_=yt[:],
                        )
    return nc


def make_in_maps(x, Wq, Wk, Wv, Wo):
    """Build the 8 per-core input maps from the full-problem inputs."""
    x = np.asarray(x, dtype=np.float32).reshape(B, C, S)
    Wq = np.asarray(Wq, np.float32)
    Wk = np.asarray(Wk, np.float32)
    Wv = np.asarray(Wv, np.float32)
    Wo = np.asarray(Wo, np.float32)
    ident = np.eye(128, dtype=np.float32)

    def f16(a):
        return np.ascontiguousarray(a).astype(np.float16)

    in_maps = []
    for core in range(N_CORES):
        b, g = divmod(core, 2)
        lo, hi = g * 256, (g + 1) * 256
        wa = np.concatenate([Wq[lo:hi], Wk[lo:hi], Wv[lo:hi]], axis=0).T.copy()
        wo_loc = np.concatenate(
            [Wo[:, lo:hi].T, Wo[:, C + lo : C + hi].T], axis=0
        ).copy()
        in_maps.append(
            {
                "x": f16(x[b]),
                "waT": f16(wa),
                "woT": f16(wo_loc),
                "ident": f16(ident),
            }
        )
    return in_maps


def combine_results(results):
    """results: list of 8 dicts with 'y' (512, S) fp32 -> full (4,512,128,128)."""
    y = np.empty((B, C, H, W), np.float32)
    for b in range(B):
        y[b] = (
            results[2 * b]["y"].astype(np.float32)
            + results[2 * b + 1]["y"].astype(np.float32)
        ).reshape(C, H, W)
    return y


_NC_CACHE = None


def get_nc():
    global _NC_CACHE
    if _NC_CACHE is None:
        _NC_CACHE = build_nc()
    return _NC_CACHE


def kernel(x, Wq, Wk, Wv, Wo):
    nc = get_nc()
    in_maps = make_in_maps(x, Wq, Wk, Wv, Wo)
    res = run_bass_kernel_spmd(nc, in_maps, list(range(N_CORES)), trace=False)
    return combine_results(res.results)
